# revision 1
# baseline (speedup 1.0000x reference)
"""Blockdiag butterfly (Monarch) linear on 8 TRN2 NeuronCores.

Math (see reference): x:[B,4096] f32, w1:[4,192,1024], w2:[4,1024,192], bias:[4096]
  stage1: out1[b,k,q] = sum_p x[b, k*1024+p] * w1[k,q,p]          (q = l*48+j)
  block transpose: out1t[b,l,r] = out1[b,k,l*48+j], r = k*48+j
  stage2: out[b, l*1024+s] = sum_r out1t[b,l,r] * w2[l,s,r] + bias

Sharding: pure data-parallel over the batch dim (2048 tokens/core),
weights replicated.  x is transposed host-side so the device never has
to transpose fp32; both matmul stages then contract over the partition
dim with no on-chip transposes: stage 1 is weight-stationary producing
psum[q, b], which is exactly the [r, b] orientation stage 2 needs as
its stationary operand.

The per-block width 48 is zero-padded to 64 host-side, so the
intermediate lives in clean 64-row partition groups and the
stage1->stage2 block transpose becomes aligned [64, bt] psum->SBUF
copies.  Feature flags (bisectable):
  PAIR: stage-2 second contraction pass as row-tiled concurrent K=64
        pairs (l-even k2-half rows 0..63 with l-odd k3-half rows
        64..127) in different psum banks.
  BIAS_IN_MM: plant bias in w2t row 255 + constant-1.0 row in out1t,
        making the psum evacuation a plain copy split DVE/ACT.
"""

import numpy as np

NB1, NB2, B1 = 4, 4, 48
B1P = 64
IN_F, OUT_F = 4096, 4096
BATCH = 16384
N_CORES = 8
B_LOCAL = BATCH // N_CORES
P = 128
NQP = NB2 * B1P              # 256
NRP = NB1 * B1P              # 256
PC = IN_F // NB1 // P        # 8
BT = 512
NBT = B_LOCAL // BT          # 4
S = OUT_F // NB2             # 1024

# Row-tiled concurrent K=64 pairs (tile_position) measured correct in
# isolation but hang the device in the full kernel -- keep disabled.
PAIR = False
BIAS_IN_MM = True

_CACHE = {}


def _emit(nc, xt, w1t, w2t, bias, out, reps=1):
    import concourse.mybir as mybir
    import concourse.tile as tile

    f32 = mybir.dt.float32

    xt_v = xt.rearrange("(k pc pi) b -> pi k pc b", k=NB1, pc=PC, pi=P)
    w1_v = w1t.rearrange("k (pc pi) q -> pi k pc q", pc=PC, pi=P)
    w2_v = w2t.rearrange("l (rc ri) s -> ri l rc s", rc=NRP // P, ri=P)

    with tile.TileContext(nc) as tc:
        with (
            tc.tile_pool(name="consts", bufs=1) as consts,
            tc.tile_pool(name="xin", bufs=2) as xin,
            tc.tile_pool(name="mid", bufs=2) as mid,
            tc.tile_pool(name="outp", bufs=4) as outp,
            tc.tile_pool(name="ps1", bufs=2, space="PSUM") as ps1,
            tc.tile_pool(name="ps2", bufs=4, space="PSUM") as ps2,
        ):
            w1_sb = consts.tile([P, NB1, PC, NQP], f32)
            nc.sync.dma_start(w1_sb[:], w1_v)
            w2_sb = consts.tile([P, NB2, NRP // P, S], f32)
            nc.sync.dma_start(w2_sb[:], w2_v)
            if not BIAS_IN_MM:
                bias_sb = consts.tile([P, OUT_F], f32)
                nc.sync.dma_start(bias_sb[:], bias.partition_broadcast(P))
            else:
                # keep the bias input alive so the NEFF keeps the tensor
                bias_sb = consts.tile([1, OUT_F], f32)
                nc.sync.dma_start(bias_sb[:], bias[None, :])

            for bt in range(NBT * reps):
                bt = bt % NBT
                bsl = slice(bt * BT, (bt + 1) * BT)
                # ---- stage 1 ----
                o1 = [
                    mid.tile([P, NB2, BT], f32, tag="o1a", name="o1a"),
                    mid.tile([P, NB2, BT], f32, tag="o1b", name="o1b"),
                ]
                for k in range(NB1):
                    xk = xin.tile([P, PC, BT], f32, tag="xk", name="xk")
                    for pc in range(PC):
                        nc.sync.dma_start(xk[:, pc, :], xt_v[:, k, pc, bsl])
                    pq = ps1.tile([P, 2, BT], f32, tag="pq", name="pq")
                    for qc in range(2):
                        for pc in range(PC):
                            nc.tensor.matmul(
                                pq[:, qc, :],
                                w1_sb[:, k, pc, qc * P:(qc + 1) * P],
                                xk[:, pc, :],
                                start=(pc == 0), stop=(pc == PC - 1),
                            )
                    if BIAS_IN_MM and k == NB1 - 1:
                        for l in range(NB2):
                            nc.gpsimd.memset(o1[1][96:128, l, :], 1.0)
                        for l in range(NB2):
                            nc.any.tensor_copy(
                                out=o1[1][64:64 + B1, l, :],
                                in_=pq[(l % 2) * B1P:(l % 2) * B1P + B1, l // 2, :],
                            )
                    else:
                        for l in range(NB2):
                            nc.any.tensor_copy(
                                out=o1[k // 2][(k % 2) * B1P:(k % 2 + 1) * B1P, l, :],
                                in_=pq[(l % 2) * B1P:(l % 2 + 1) * B1P, l // 2, :],
                            )
                # ---- stage 2 ----
                for bi in range(BT // P):
                    b0 = bt * BT + bi * P
                    bloc = slice(bi * P, (bi + 1) * P)
                    for lp in range(0, NB2, 2):
                        pss = {}
                        for sh in range(S // 512):
                            ssl = slice(sh * 512, (sh + 1) * 512)
                            for l in (lp, lp + 1):
                                ps = ps2.tile([P, 512], f32, tag="ps2", name="ps2")
                                pss[(l, sh)] = ps
                                nc.tensor.matmul(
                                    ps[:], o1[0][:, l, bloc], w2_sb[:, l, 0, ssl],
                                    start=True, stop=False,
                                )
                            if PAIR:
                                nc.tensor.matmul(
                                    pss[(lp, sh)][:], o1[1][0:64, lp, bloc],
                                    w2_sb[0:64, lp, 1, ssl],
                                    start=False, stop=False, tile_position=(0, 0),
                                )
                                nc.tensor.matmul(
                                    pss[(lp + 1, sh)][:], o1[1][64:128, lp + 1, bloc],
                                    w2_sb[64:128, lp + 1, 1, ssl],
                                    start=False, stop=False, tile_position=(64, 0),
                                )
                                nc.tensor.matmul(
                                    pss[(lp + 1, sh)][:], o1[1][0:64, lp + 1, bloc],
                                    w2_sb[0:64, lp + 1, 1, ssl],
                                    start=False, stop=True, tile_position=(0, 0),
                                )
                                nc.tensor.matmul(
                                    pss[(lp, sh)][:], o1[1][64:128, lp, bloc],
                                    w2_sb[64:128, lp, 1, ssl],
                                    start=False, stop=True, tile_position=(64, 0),
                                )
                            else:
                                for l in (lp, lp + 1):
                                    nc.tensor.matmul(
                                        pss[(l, sh)][:], o1[1][:, l, bloc],
                                        w2_sb[:, l, 1, ssl],
                                        start=False, stop=True,
                                    )
                        for l in (lp, lp + 1):
                            ob = outp.tile([P, S], f32, tag="ob", name="ob")
                            for sh in range(S // 512):
                                ssl = slice(sh * 512, (sh + 1) * 512)
                                if BIAS_IN_MM:
                                    if l % 2 == 1:
                                        nc.scalar.copy(ob[:, ssl], pss[(l, sh)][:])
                                    else:
                                        nc.vector.tensor_copy(
                                            out=ob[:, ssl], in_=pss[(l, sh)][:])
                                else:
                                    nc.vector.tensor_add(
                                        out=ob[:, ssl], in0=pss[(l, sh)][:],
                                        in1=bias_sb[:, l * S + sh * 512:
                                                    l * S + (sh + 1) * 512],
                                    )
                            nc.sync.dma_start(
                                out[b0:b0 + P, l * S:(l + 1) * S], ob[:])


def _build(reps=1):
    import concourse.bacc as bacc
    import concourse.mybir as mybir

    # Bacc (not plain Bass): its compile() legalizes semaphore waits
    # (move_matmul_waits_to_ldweights + generate_event_semaphores) --
    # walrus allows at most one sync wait per instruction.
    nc = bacc.Bacc(name=f"bfly_r{reps}")
    xt = nc.dram_tensor("xt", [IN_F, B_LOCAL], mybir.dt.float32, kind="ExternalInput")
    w1t = nc.dram_tensor("w1t", [NB1, IN_F // NB1, NQP], mybir.dt.float32, kind="ExternalInput")
    w2t = nc.dram_tensor("w2t", [NB2, NRP, S], mybir.dt.float32, kind="ExternalInput")
    bias = nc.dram_tensor("bias", [OUT_F], mybir.dt.float32, kind="ExternalInput")
    out = nc.dram_tensor("out", [B_LOCAL, OUT_F], mybir.dt.float32, kind="ExternalOutput")
    _emit(nc, xt[:], w1t[:], w2t[:], bias[:], out[:], reps=reps)
    nc.compile()
    return nc


def get_nc(reps=1):
    key = ("nc", reps, PAIR, BIAS_IN_MM)
    if key not in _CACHE:
        _CACHE[key] = _build(reps)
    return _CACHE[key]


def prep_weights(w1_bfly, w2_bfly, bias):
    """Pad the per-block width 48 -> 64 and transpose for the device
    layout; optionally plant bias in w2t's last padding row."""
    w1t = np.zeros((NB1, IN_F // NB1, NQP), dtype=np.float32)
    w1t_v = w1t.reshape(NB1, IN_F // NB1, NB2, B1P)
    w1t_v[:, :, :, :B1] = (
        w1_bfly.transpose(0, 2, 1).reshape(NB1, IN_F // NB1, NB2, B1)
    )
    w2t = np.zeros((NB2, NRP, S), dtype=np.float32)
    w2t_v = w2t.reshape(NB2, NB1, B1P, S)
    w2t_v[:, :, :B1, :] = (
        w2_bfly.transpose(0, 2, 1).reshape(NB2, NB1, B1, S)
    )
    if BIAS_IN_MM:
        w2t[:, NRP - 1, :] = np.asarray(bias, np.float32).reshape(NB2, S)
    return w1t, w2t


def _prep_inputs(x, w1_bfly, w2_bfly, bias):
    bias = np.ascontiguousarray(np.asarray(bias, np.float32))
    w1t, w2t = prep_weights(w1_bfly, w2_bfly, bias)
    in_maps = []
    for c in range(N_CORES):
        xs = np.ascontiguousarray(x[c * B_LOCAL:(c + 1) * B_LOCAL].T)
        in_maps.append({"xt": xs, "w1t": w1t, "w2t": w2t, "bias": bias})
    return in_maps


def kernel(x, w1_bfly, w2_bfly, bias):
    from concourse.bass_utils import run_bass_kernel_spmd

    nc = get_nc()
    in_maps = _prep_inputs(np.asarray(x), np.asarray(w1_bfly),
                           np.asarray(w2_bfly), np.asarray(bias))
    res = run_bass_kernel_spmd(nc, in_maps, list(range(N_CORES)), trace=False)
    return np.concatenate([res.results[c]["out"] for c in range(N_CORES)], axis=0)



# revision 6
# speedup vs baseline: 1614.3233x; 1614.3233x over previous
"""Blockdiag butterfly (Monarch) linear on 8 TRN2 NeuronCores.

Math (see reference): x:[B,4096] f32, w1:[4,192,1024], w2:[4,1024,192], bias:[4096]
  stage1: out1[b,k,q] = sum_p x[b, k*1024+p] * w1[k,q,p]          (q = l*48+j)
  block transpose: out1t[b,l,r] = out1[b,k,l*48+j], r = k*48+j
  stage2: out[b, l*1024+s] = sum_r out1t[b,l,r] * w2[l,s,r] + bias

Sharding: pure data-parallel over the batch dim (2048 tokens/core),
weights replicated.  x is transposed and cast to bf16 host-side, so the
device never transposes and every matmul runs at the 1-cycle/row bf16
rate (fp32 is 4 cycles/row on TRN2) while halving HBM traffic.  Both
stages contract over the partition dim with no on-chip transposes:
stage 1 is weight-stationary producing psum[q, b], which is exactly the
[r, b] orientation stage 2 needs as its stationary operand.

The per-block width 48 is zero-padded to 64 host-side, so the
intermediate lives in clean 64-row partition groups and the
stage1->stage2 block transpose becomes aligned psum->SBUF copies
(cast to bf16 on the way).  Bias is planted in w2's last padding row
(255) against a constant-1.0 row memset into the intermediate, making
psum evacuation a plain cast-copy split across DVE/ACT.  Output is
written bf16 and expanded to f32 host-side (rel tolerance is 2e-2;
bf16 end-to-end error is ~1e-3 of output scale).
"""

import numpy as np

NB1, NB2, B1 = 4, 4, 48
B1P = 64
IN_F, OUT_F = 4096, 4096
BATCH = 16384
N_CORES = 8
B_LOCAL = BATCH // N_CORES
P = 128
NQP = NB2 * B1P              # 256
NRP = NB1 * B1P              # 256
PC = IN_F // NB1 // P        # 8
BT = 512
NBT = B_LOCAL // BT          # 4
S = OUT_F // NB2             # 1024

_CACHE = {}


def _np_bf16():
    import ml_dtypes
    return ml_dtypes.bfloat16


def _emit(nc, xt, w1t, w2t, bias, out, reps=1):
    import concourse.mybir as mybir
    import concourse.tile as tile

    f32 = mybir.dt.float32
    bf16 = mybir.dt.bfloat16

    xt_v = xt.rearrange("(k pc pi) b -> pi k pc b", k=NB1, pc=PC, pi=P)
    w1_v = w1t.rearrange("k (pc pi) q -> pi k pc q", pc=PC, pi=P)
    w2_v = w2t.rearrange("l (rc ri) s -> ri l rc s", rc=NRP // P, ri=P)

    with tile.TileContext(nc) as tc:
        with (
            tc.tile_pool(name="consts", bufs=1) as consts,
            tc.tile_pool(name="xin", bufs=2) as xin,
            tc.tile_pool(name="mid", bufs=2) as mid,
            tc.tile_pool(name="outp", bufs=2) as outp,
            tc.tile_pool(name="ps1", bufs=2, space="PSUM") as ps1,
            tc.tile_pool(name="ps2", bufs=4, space="PSUM") as ps2,
        ):
            # weights go out on the ACT/DVE HWDGE rings so the SP ring is
            # free for the first x-tile load
            w1_sb = consts.tile([P, NB1, PC, NQP], bf16)
            nc.scalar.dma_start(w1_sb[:], w1_v)
            w2_sb = consts.tile([P, NB2, NRP // P, S], bf16)
            nc.scalar.dma_start(w2_sb[:], w2_v)
            # keep the bias input alive so the NEFF keeps the tensor
            bias_sb = consts.tile([1, OUT_F], f32)
            nc.scalar.dma_start(bias_sb[:], bias[None, :])

            for bt in range(NBT * reps):
                bt = bt % NBT
                bsl = slice(bt * BT, (bt + 1) * BT)
                # ---- stage 1 ----
                xk = xin.tile([P, NB1, PC, BT], bf16, tag="xk", name="xk")
                nc.sync.dma_start(xk[:], xt_v[:, :, :, bsl])
                o1 = [
                    mid.tile([P, NB2, BT], bf16, tag="o1a", name="o1a"),
                    mid.tile([P, NB2, BT], bf16, tag="o1b", name="o1b"),
                ]
                # constant-1.0 rows facing w2's bias row: gpsimd needs a
                # 32-aligned start partition, so set 96:128 and let the k=3
                # copy overwrite 96:112 with real data; rows 112:126 face
                # w2 zero-padding rows and 127 faces the bias row
                nc.gpsimd.memset(o1[1][96:2 * B1P, :, :], 1.0)
                for k in range(NB1):
                    pq = ps1.tile([P, 2, BT], f32, tag="pq", name="pq")
                    for qc in range(2):
                        for pc in range(PC):
                            nc.tensor.matmul(
                                pq[:, qc, :],
                                w1_sb[:, k, pc, qc * P:(qc + 1) * P],
                                xk[:, k, pc, :],
                                start=(pc == 0), stop=(pc == PC - 1),
                            )
                    half = k // 2
                    r0 = (k % 2) * B1P
                    nrow = B1P if not (half == 1 and k % 2 == 1) else B1
                    for l in range(NB2):
                        dst = o1[half][r0:r0 + nrow, l, :]
                        src = pq[(l % 2) * B1P:(l % 2) * B1P + nrow, l // 2, :]
                        if l % 2 == 0:
                            nc.vector.tensor_copy(out=dst, in_=src)
                        else:
                            nc.scalar.copy(dst, src)
                # ---- stage 2 ----
                for bi in range(BT // P):
                    b0 = bt * BT + bi * P
                    bloc = slice(bi * P, (bi + 1) * P)
                    ob = outp.tile([P, NB2, S], bf16, tag="ob", name="ob")
                    for l in range(NB2):
                        for sh in range(S // 512):
                            ssl = slice(sh * 512, (sh + 1) * 512)
                            ps = ps2.tile([P, 512], f32, tag="ps2", name="ps2")
                            nc.tensor.matmul(
                                ps[:], o1[0][:, l, bloc], w2_sb[:, l, 0, ssl],
                                start=True, stop=False,
                            )
                            nc.tensor.matmul(
                                ps[:], o1[1][:, l, bloc], w2_sb[:, l, 1, ssl],
                                start=False, stop=True,
                            )
                            if (l + sh) % 2 == 0:
                                nc.vector.tensor_copy(out=ob[:, l, ssl], in_=ps[:])
                            else:
                                nc.scalar.copy(ob[:, l, ssl], ps[:])
                    # output stores ride the otherwise-idle Pool (SWDGE)
                    # queue so they don't serialize behind x loads on SP
                    nc.gpsimd.dma_start(out[b0:b0 + P, :], ob[:])


def _build(reps=1):
    import concourse.bacc as bacc
    import concourse.mybir as mybir

    # Bacc (not plain Bass): its compile() legalizes semaphore waits
    # (move_matmul_waits_to_ldweights + generate_event_semaphores) --
    # walrus allows at most one sync wait per instruction.
    nc = bacc.Bacc(name=f"bfly_r{reps}")
    bf16 = mybir.dt.bfloat16
    xt = nc.dram_tensor("xt", [IN_F, B_LOCAL], bf16, kind="ExternalInput")
    w1t = nc.dram_tensor("w1t", [NB1, IN_F // NB1, NQP], bf16, kind="ExternalInput")
    w2t = nc.dram_tensor("w2t", [NB2, NRP, S], bf16, kind="ExternalInput")
    bias = nc.dram_tensor("bias", [OUT_F], mybir.dt.float32, kind="ExternalInput")
    out = nc.dram_tensor("out", [B_LOCAL, OUT_F], bf16, kind="ExternalOutput")
    _emit(nc, xt[:], w1t[:], w2t[:], bias[:], out[:], reps=reps)
    nc.compile()
    return nc


def get_nc(reps=1):
    key = ("nc", reps)
    if key not in _CACHE:
        _CACHE[key] = _build(reps)
    return _CACHE[key]


def prep_weights(w1_bfly, w2_bfly, bias):
    """Pad the per-block width 48 -> 64, transpose for the device layout,
    cast to bf16, and plant bias in w2t's last padding row."""
    bf16 = _np_bf16()
    w1t = np.zeros((NB1, IN_F // NB1, NQP), dtype=np.float32)
    w1t_v = w1t.reshape(NB1, IN_F // NB1, NB2, B1P)
    w1t_v[:, :, :, :B1] = (
        np.asarray(w1_bfly, np.float32)
        .transpose(0, 2, 1).reshape(NB1, IN_F // NB1, NB2, B1)
    )
    w2t = np.zeros((NB2, NRP, S), dtype=np.float32)
    w2t_v = w2t.reshape(NB2, NB1, B1P, S)
    w2t_v[:, :, :B1, :] = (
        np.asarray(w2_bfly, np.float32)
        .transpose(0, 2, 1).reshape(NB2, NB1, B1, S)
    )
    w2t[:, NRP - 1, :] = np.asarray(bias, np.float32).reshape(NB2, S)
    return w1t.astype(bf16), w2t.astype(bf16)


def _prep_inputs(x, w1_bfly, w2_bfly, bias):
    bf16 = _np_bf16()
    bias = np.ascontiguousarray(np.asarray(bias, np.float32))
    w1t, w2t = prep_weights(w1_bfly, w2_bfly, bias)
    xb = np.asarray(x, np.float32).astype(bf16)
    in_maps = []
    for c in range(N_CORES):
        xs = np.ascontiguousarray(xb[c * B_LOCAL:(c + 1) * B_LOCAL].T)
        in_maps.append({"xt": xs, "w1t": w1t, "w2t": w2t, "bias": bias})
    return in_maps


def kernel(x, w1_bfly, w2_bfly, bias):
    from concourse.bass_utils import run_bass_kernel_spmd

    nc = get_nc()
    in_maps = _prep_inputs(np.asarray(x), np.asarray(w1_bfly),
                           np.asarray(w2_bfly), np.asarray(bias))
    res = run_bass_kernel_spmd(nc, in_maps, list(range(N_CORES)), trace=False)
    return np.concatenate(
        [np.asarray(res.results[c]["out"], np.float32) for c in range(N_CORES)],
        axis=0)
